# revision 16
# baseline (speedup 1.0000x reference)
"""Trainium2 Bass kernel for nn_GaussianBlur: depthwise 2D conv, 71x71 kernel,
x [16,3,512,512] fp32.

Strategy (v3):
  - Separable rank-1 2D kernel: conv = Gh @ X @ Gw^T, Gh/Gw banded 512x512
    Toeplitz blur matrices.
  - Each blur matrix is numerically rank<=64 (spectrum ~ exp(-s^2 w^2/2)):
    Gh ~= Uh @ Dh with Dh [64,512], Uh [512,64] from SVD (rel err ~9e-4).
  - Four chained matmul passes per (n,c) slice, no transposes needed:
        A: P[w,m]  = sum_h X[h,w]  DhT[h,m]    16 MM N=64  (lhsT = X tiles)
        B: Q[m,n]  = sum_w P[w,m]  DwT[w,n]     4 MM N=64
        C: R[n,h'] = sum_m Q[m,n]  UhT[m,h']    1 MM N=512
        D: Y[h',w']= sum_n R[n,h'] UwT[n,w']    4 MM N=512
  - bf16 in HBM (halves DMA; 1 cycle/row at any N), fp32 PSUM accumulate.
  - PE kept continuously busy (HAM stays at 2.4GHz): slices are software-
    pipelined A/B two slices ahead, C one ahead, plus warm-up matmuls during
    the initial DMA fill. x/y are host-swizzled to partition-major so each
    slice is one 512KB DMA with 4KB-contiguous descriptors.
  - Data parallel: 48 (n,c) slices, 6 per core across 8 NeuronCores.
"""

import sys

sys.path.insert(0, "/opt/trn_rl_repo")

from contextlib import ExitStack

import numpy as np
import ml_dtypes

import concourse.bass as bass
import concourse.tile as tile
from concourse import bacc, mybir
from concourse.bass import ts
from concourse.bass_utils import run_bass_kernel_spmd


def _ensure_axon_hooks():
    """Some agent containers ship an ``antenv`` without ``axon_hooks``;
    bass_utils hard-imports it when BASS_TRACE is set under axon. Provide
    the module (and the ctypes NTFF hook, when libaxon supports it) so
    tracing works instead of crashing."""
    try:
        import antenv.axon_hooks  # noqa: F401

        return
    except ImportError:
        pass
    import types

    mod = types.ModuleType("antenv.axon_hooks")
    _state = {"hook": None}
    mod.set_axon_ntff_profile_hook = lambda h: _state.__setitem__("hook", h)
    mod.get_axon_ntff_profile_hook = lambda: _state["hook"]
    sys.modules["antenv.axon_hooks"] = mod
    try:
        import antenv

        antenv.axon_hooks = mod
    except ImportError:
        pass
    try:
        from trn_agent_boot.trn_boot import _ntff_profile_via_ctypes

        so = "/opt/axon/libaxon_pjrt.so"
        import os

        if os.path.exists(so):
            hook = _ntff_profile_via_ctypes(so)
            if hook is not None:
                _state["hook"] = hook
    except Exception:
        pass


_ensure_axon_hooks()

BF = ml_dtypes.bfloat16

N_CORES = 8
H = W = 512
PT = 128          # partition tile
NT = H // PT      # 4 tiles per 512 dim
S = 6             # slices per core: 16*3 / 8
PAD = 35
KS = 71
R = 64            # numerical rank of the 512x512 blur matrix
N_WARM = 100      # PE warm-up matmuls during initial DMA fill (~5us: HAM
                  # needs ~3.4us of sustained PE activity to unthrottle)

_kernel_cache = {}


def _build_bass():
    f32 = mybir.dt.float32
    bf16 = mybir.dt.bfloat16

    nc = bacc.Bacc(name="gaussblur64")
    # x/y are partition-major: [s, p, tk, w] = slice s, row tk*128+p, col w
    x_d = nc.dram_tensor("x", [S, PT, NT, W], bf16, kind="ExternalInput")
    # c128[p, which(dht=0,dwt=1), tk, m] = {DhT,DwT}[128*tk + p, m]
    c128_d = nc.dram_tensor("c128", [PT, 2, NT, R], bf16, kind="ExternalInput")
    # c64[m, which(uht=0,uwt=1), j] = {UhT,UwT}[m, j]
    c64_d = nc.dram_tensor("c64", [R, 2, W], bf16, kind="ExternalInput")
    y_d = nc.dram_tensor("y", [S, PT, NT, W], bf16, kind="ExternalOutput")

    with tile.TileContext(nc) as tc, ExitStack() as ctx:
        cpool = ctx.enter_context(tc.tile_pool(name="const", bufs=1))
        xp = ctx.enter_context(tc.tile_pool(name="xp", bufs=S))
        pp = ctx.enter_context(tc.tile_pool(name="pp", bufs=2))
        qp = ctx.enter_context(tc.tile_pool(name="qp", bufs=3))
        rp = ctx.enter_context(tc.tile_pool(name="rp", bufs=3))
        yp = ctx.enter_context(tc.tile_pool(name="yp", bufs=2))
        psA = ctx.enter_context(tc.tile_pool(name="psA", bufs=2, space="PSUM"))
        psB = ctx.enter_context(tc.tile_pool(name="psB", bufs=1, space="PSUM"))
        psC = ctx.enter_context(tc.tile_pool(name="psC", bufs=1, space="PSUM"))
        psD = ctx.enter_context(tc.tile_pool(name="psD", bufs=2, space="PSUM"))

        c128_t = cpool.tile([PT, 2, NT, R], bf16)
        c64_t = cpool.tile([R, 2, W], bf16)
        scratch = cpool.tile([PT, R], bf16)  # never written: warm-up operand

        x_t = [None] * S
        p_t = [None] * S
        q_t = [None] * S
        r_t = [None] * S

        def load_x(s):
            x_t[s] = xp.tile([PT, NT, W], bf16, name="x", tag="x")
            q = nc.sync if s % 2 == 0 else nc.scalar
            q.dma_start(x_t[s][:], x_d.ap()[s])

        def stage_A(s):
            p_t[s] = pp.tile([PT, NT, R], bf16, name="p", tag="p")
            for ws in range(NT):
                pa = psA.tile([PT, R], f32, name="pa", tag="pa")
                for tk in range(NT):
                    nc.tensor.matmul(
                        pa[:],
                        x_t[s][:, tk, ts(ws, PT)],
                        c128_t[:, 0, tk, :],
                        start=(tk == 0),
                        stop=(tk == NT - 1),
                    )
                nc.vector.tensor_copy(p_t[s][:, ws, :], pa[:])

        def stage_B(s):
            qb = psB.tile([R, R], f32, name="qb", tag="qb")
            for wt in range(NT):
                nc.tensor.matmul(
                    qb[:],
                    p_t[s][:, wt, :],
                    c128_t[:, 1, wt, :],
                    start=(wt == 0),
                    stop=(wt == NT - 1),
                )
            q_t[s] = qp.tile([R, R], bf16, name="q", tag="q")
            nc.vector.tensor_copy(q_t[s][:], qb[:])

        def stage_C(s):
            rc = psC.tile([R, W], f32, name="rc", tag="rc")
            nc.tensor.matmul(rc[:], q_t[s][:], c64_t[:, 0, :], start=True, stop=True)
            r_t[s] = rp.tile([R, W], bf16, name="r", tag="r")
            nc.scalar.copy(r_t[s][:], rc[:])

        def stage_D(s, fine=False):
            y_t = yp.tile([PT, NT, W], bf16, name="y", tag="y")
            yq = nc.sync if s % 2 == 0 else nc.scalar
            for half in range(2):
                yd = psD.tile([PT, 2, W], f32, name="yd", tag="yd")
                for j in range(2):
                    hs = 2 * half + j
                    nc.tensor.matmul(
                        yd[:, j, :],
                        r_t[s][:, ts(hs, PT)],
                        c64_t[:, 1, :],
                        start=True,
                        stop=True,
                    )
                    if fine:
                        eng = nc.vector.tensor_copy if j == 0 else nc.scalar.copy
                        eng(y_t[:, hs, :], yd[:, j, :])
                        yq.dma_start(y_d.ap()[s, :, hs, :], y_t[:, hs, :])
                if not fine:
                    if half == 0:
                        nc.vector.tensor_copy(y_t[:, 0:2, :], yd[:])
                    else:
                        nc.scalar.copy(y_t[:, 2:4, :], yd[:])
                    yq.dma_start(
                        y_d.ap()[s, :, ts(half, 2), :], y_t[:, ts(half, 2), :]
                    )

        # Prologue: first x + consts dispatched immediately, remaining x
        # DMAs queued upfront across both HWDGE rings.
        load_x(0)
        nc.scalar.dma_start(c128_t[:], c128_d.ap()[:])
        nc.scalar.dma_start(c64_t[:], c64_d.ap()[:])
        for s in range(1, S):
            load_x(s)

        # Warm-up matmuls on uninitialized scratch: keep the PE active during
        # the initial DMA fill so HAM unthrottles before real work arrives.
        nc.vector.memset(scratch[:], 1.0)
        wu = psA.tile([PT, R], f32, name="wu", tag="pa")

        def warm(n):
            # Filler matmuls on scratch: keep the PE active so HAM never
            # re-throttles to 1.2GHz while waiting on DMA.
            for _ in range(n):
                nc.tensor.matmul(
                    wu[:R, :], scratch[:], scratch[:, :R], start=True,
                    stop=True, skip_group_check=True,
                )

        warm(N_WARM)

        # Software pipeline: A/B two slices ahead, C one ahead, D current.
        # C(s+1) goes before D(s) so D(s+1) never waits on the rc copy.
        stage_A(0)
        stage_B(0)
        warm(40)
        stage_A(1)
        stage_B(1)
        warm(40)
        stage_C(0)
        for s in range(S):
            if s + 2 < S:
                warm(16)
                stage_A(s + 2)
                stage_B(s + 2)
            if s + 1 < S:
                stage_C(s + 1)
            stage_D(s, fine=(s == S - 1))

    nc.compile()
    return nc


def _band_corr(taps: np.ndarray, n: int) -> np.ndarray:
    """G[i, j] = taps[j - i + PAD]: cross-correlation (matches NN conv)."""
    M = np.zeros((n, n), np.float64)
    idx = np.arange(n)
    for d in range(-PAD, PAD + 1):
        i = idx[(idx + d >= 0) & (idx + d < n)]
        M[i, i + d] = taps[d + PAD]
    return M


def _factors(taps: np.ndarray):
    """Rank-R factorization G ~= U_ @ D_ of the 1D blur matrix for `taps`."""
    G = _band_corr(taps, H)
    U, Sv, Vt = np.linalg.svd(G)
    D_ = (np.sqrt(Sv[:R])[:, None] * Vt[:R]).astype(np.float32)  # [R, 512]
    U_ = (U[:, :R] * np.sqrt(Sv[:R])).astype(np.float32)         # [512, R]
    return D_, U_


def kernel(x: np.ndarray, kernel: np.ndarray) -> np.ndarray:
    x = np.asarray(x, dtype=np.float32)
    k2d = np.asarray(kernel, dtype=np.float64)
    n, c, h, w = x.shape
    assert (h, w) == (H, W) and k2d.shape == (KS, KS)

    # Separable decomposition; the Gaussian (and all-ones) kernel is rank-1.
    U, Sv, Vt = np.linalg.svd(k2d)
    assert Sv[1] <= Sv[0] * 1e-6, "kernel not rank-1 separable"
    kx = Sv[0] * U[:, 0]  # taps along H
    ky = Vt[0]            # taps along W

    Dh, Uh = _factors(kx)
    Dw, Uw = _factors(ky)

    c128 = np.empty((PT, 2, NT, R), np.float32)
    c128[:, 0] = Dh.T.reshape(NT, PT, R).transpose(1, 0, 2)  # DhT[128*tk+p, m]
    c128[:, 1] = Dw.T.reshape(NT, PT, R).transpose(1, 0, 2)
    c64 = np.empty((R, 2, W), np.float32)
    c64[:, 0] = Uh.T  # UhT[m, h']
    c64[:, 1] = Uw.T  # UwT[n, w']
    c128 = c128.astype(BF)
    c64 = c64.astype(BF)

    if "k" not in _kernel_cache:
        _kernel_cache["k"] = _build_bass()
    nc = _kernel_cache["k"]

    # Partition-major swizzle: x_swz[s, p, tk, w] = x[s, tk*128+p, w]
    xr = x.reshape(n * c, NT, PT, W).transpose(0, 2, 1, 3).astype(BF)
    per = xr.shape[0] // N_CORES
    in_maps = [
        {
            "x": np.ascontiguousarray(xr[ci * per : (ci + 1) * per]),
            "c128": c128,
            "c64": c64,
        }
        for ci in range(N_CORES)
    ]
    res = run_bass_kernel_spmd(nc, in_maps, core_ids=list(range(N_CORES)))
    global last_results
    last_results = res
    y = np.concatenate([res.results[ci]["y"] for ci in range(N_CORES)], axis=0)
    # y[s, p, hs, w] -> [s, hs*128+p, w]
    y = y.transpose(0, 2, 1, 3).reshape(n, c, h, w).astype(np.float32)
    return y


last_results = None


# revision 19
# speedup vs baseline: 1.0308x; 1.0308x over previous
"""Trainium2 Bass kernel for nn_GaussianBlur: depthwise 2D conv, 71x71 kernel,
x [16,3,512,512] fp32.

Strategy (v3):
  - Separable rank-1 2D kernel: conv = Gh @ X @ Gw^T, Gh/Gw banded 512x512
    Toeplitz blur matrices.
  - Each blur matrix is numerically rank<=64 (spectrum ~ exp(-s^2 w^2/2)):
    Gh ~= Uh @ Dh with Dh [64,512], Uh [512,64] from SVD (rel err ~9e-4).
  - Four chained matmul passes per (n,c) slice, no transposes needed:
        A: P[w,m]  = sum_h X[h,w]  DhT[h,m]    16 MM N=64  (lhsT = X tiles)
        B: Q[m,n]  = sum_w P[w,m]  DwT[w,n]     4 MM N=64
        C: R[n,h'] = sum_m Q[m,n]  UhT[m,h']    1 MM N=512
        D: Y[h',w']= sum_n R[n,h'] UwT[n,w']    4 MM N=512
  - bf16 in HBM (halves DMA; 1 cycle/row at any N), fp32 PSUM accumulate.
  - PE kept continuously busy (HAM stays at 2.4GHz): slices are software-
    pipelined A/B two slices ahead, C one ahead, plus warm-up matmuls during
    the initial DMA fill. x/y are host-swizzled to partition-major so each
    slice is one 512KB DMA with 4KB-contiguous descriptors.
  - Data parallel: 48 (n,c) slices, 6 per core across 8 NeuronCores.
"""

import sys

sys.path.insert(0, "/opt/trn_rl_repo")

from contextlib import ExitStack

import numpy as np
import ml_dtypes

import concourse.bass as bass
import concourse.tile as tile
from concourse import bacc, mybir
from concourse.bass import ts
from concourse.bass_utils import run_bass_kernel_spmd


def _ensure_axon_hooks():
    """Some agent containers ship an ``antenv`` without ``axon_hooks``;
    bass_utils hard-imports it when BASS_TRACE is set under axon. Provide
    the module (and the ctypes NTFF hook, when libaxon supports it) so
    tracing works instead of crashing."""
    try:
        import antenv.axon_hooks  # noqa: F401

        return
    except ImportError:
        pass
    import types

    mod = types.ModuleType("antenv.axon_hooks")
    _state = {"hook": None}
    mod.set_axon_ntff_profile_hook = lambda h: _state.__setitem__("hook", h)
    mod.get_axon_ntff_profile_hook = lambda: _state["hook"]
    sys.modules["antenv.axon_hooks"] = mod
    try:
        import antenv

        antenv.axon_hooks = mod
    except ImportError:
        pass
    try:
        from trn_agent_boot.trn_boot import _ntff_profile_via_ctypes

        so = "/opt/axon/libaxon_pjrt.so"
        import os

        if os.path.exists(so):
            hook = _ntff_profile_via_ctypes(so)
            if hook is not None:
                _state["hook"] = hook
    except Exception:
        pass


_ensure_axon_hooks()

BF = ml_dtypes.bfloat16

N_CORES = 8
H = W = 512
PT = 128          # partition tile
NT = H // PT      # 4 tiles per 512 dim
S = 6             # slices per core: 16*3 / 8
PAD = 35
KS = 71
R = 64            # numerical rank of the 512x512 blur matrix
# PE warm-up matmuls: HAM unthrottles (1.2->2.4GHz) only after ~3.4us of
# UNINTERRUPTED PE activity, and re-throttles after ~3.4us of continuous
# idle. The two fill blocks bridge the PE from engine-start (~7.5us) past
# the x0 DMA semaphore (~12us) and the x2 semaphore (~17.5us) so the
# pipeline never has a >3.4us stall. Overshoot costs 29ns/matmul; a stall
# costs a 2x clock penalty on everything after it.
N_WARM1 = 150
N_WARM2 = 170

_kernel_cache = {}


def _build_bass():
    f32 = mybir.dt.float32
    bf16 = mybir.dt.bfloat16

    nc = bacc.Bacc(name="gaussblur64")
    # x/y are partition-major: [s, p, tk, w] = slice s, row tk*128+p, col w
    x_d = nc.dram_tensor("x", [S, PT, NT, W], bf16, kind="ExternalInput")
    # c128[p, which(dht=0,dwt=1), tk, m] = {DhT,DwT}[128*tk + p, m]
    c128_d = nc.dram_tensor("c128", [PT, 2, NT, R], bf16, kind="ExternalInput")
    # c64[m, which(uht=0,uwt=1), j] = {UhT,UwT}[m, j]
    c64_d = nc.dram_tensor("c64", [R, 2, W], bf16, kind="ExternalInput")
    y_d = nc.dram_tensor("y", [S, PT, NT, W], bf16, kind="ExternalOutput")

    with tile.TileContext(nc) as tc, ExitStack() as ctx:
        cpool = ctx.enter_context(tc.tile_pool(name="const", bufs=1))
        xp = ctx.enter_context(tc.tile_pool(name="xp", bufs=S))
        pp = ctx.enter_context(tc.tile_pool(name="pp", bufs=2))
        qp = ctx.enter_context(tc.tile_pool(name="qp", bufs=3))
        rp = ctx.enter_context(tc.tile_pool(name="rp", bufs=3))
        yp = ctx.enter_context(tc.tile_pool(name="yp", bufs=2))
        psA = ctx.enter_context(tc.tile_pool(name="psA", bufs=2, space="PSUM"))
        psB = ctx.enter_context(tc.tile_pool(name="psB", bufs=1, space="PSUM"))
        psC = ctx.enter_context(tc.tile_pool(name="psC", bufs=1, space="PSUM"))
        psD = ctx.enter_context(tc.tile_pool(name="psD", bufs=2, space="PSUM"))

        c128_t = cpool.tile([PT, 2, NT, R], bf16)
        c64_t = cpool.tile([R, 2, W], bf16)
        scratch = cpool.tile([PT, R], bf16)  # never written: warm-up operand

        x_t = [None] * S
        p_t = [None] * S
        q_t = [None] * S
        r_t = [None] * S

        def load_x(s):
            x_t[s] = xp.tile([PT, NT, W], bf16, name="x", tag="x")
            q = nc.sync if s % 2 == 0 else nc.scalar
            q.dma_start(x_t[s][:], x_d.ap()[s])

        def stage_A(s):
            p_t[s] = pp.tile([PT, NT, R], bf16, name="p", tag="p")
            for ws in range(NT):
                pa = psA.tile([PT, R], f32, name="pa", tag="pa")
                for tk in range(NT):
                    nc.tensor.matmul(
                        pa[:],
                        x_t[s][:, tk, ts(ws, PT)],
                        c128_t[:, 0, tk, :],
                        start=(tk == 0),
                        stop=(tk == NT - 1),
                    )
                nc.vector.tensor_copy(p_t[s][:, ws, :], pa[:])

        def stage_B(s):
            qb = psB.tile([R, R], f32, name="qb", tag="qb")
            for wt in range(NT):
                nc.tensor.matmul(
                    qb[:],
                    p_t[s][:, wt, :],
                    c128_t[:, 1, wt, :],
                    start=(wt == 0),
                    stop=(wt == NT - 1),
                )
            q_t[s] = qp.tile([R, R], bf16, name="q", tag="q")
            nc.vector.tensor_copy(q_t[s][:], qb[:])

        def stage_C(s):
            rc = psC.tile([R, W], f32, name="rc", tag="rc")
            nc.tensor.matmul(rc[:], q_t[s][:], c64_t[:, 0, :], start=True, stop=True)
            r_t[s] = rp.tile([R, W], bf16, name="r", tag="r")
            nc.scalar.copy(r_t[s][:], rc[:])

        def stage_D(s, fine=False):
            y_t = yp.tile([PT, NT, W], bf16, name="y", tag="y")
            yq = nc.sync if s % 2 == 0 else nc.scalar
            for half in range(2):
                yd = psD.tile([PT, 2, W], f32, name="yd", tag="yd")
                for j in range(2):
                    hs = 2 * half + j
                    nc.tensor.matmul(
                        yd[:, j, :],
                        r_t[s][:, ts(hs, PT)],
                        c64_t[:, 1, :],
                        start=True,
                        stop=True,
                    )
                    if fine:
                        eng = nc.vector.tensor_copy if j == 0 else nc.scalar.copy
                        eng(y_t[:, hs, :], yd[:, j, :])
                        yq.dma_start(y_d.ap()[s, :, hs, :], y_t[:, hs, :])
                if not fine:
                    if half == 0:
                        nc.vector.tensor_copy(y_t[:, 0:2, :], yd[:])
                    else:
                        nc.scalar.copy(y_t[:, 2:4, :], yd[:])
                    yq.dma_start(
                        y_d.ap()[s, :, ts(half, 2), :], y_t[:, ts(half, 2), :]
                    )

        # Prologue: first x + consts dispatched immediately, remaining x
        # DMAs queued upfront across both HWDGE rings.
        load_x(0)
        nc.scalar.dma_start(c128_t[:], c128_d.ap()[:])
        nc.scalar.dma_start(c64_t[:], c64_d.ap()[:])
        for s in range(1, S):
            load_x(s)

        # Warm-up matmuls on uninitialized scratch: keep the PE active during
        # the initial DMA fill so HAM unthrottles before real work arrives.
        nc.vector.memset(scratch[:], 1.0)
        wu = psA.tile([PT, R], f32, name="wu", tag="pa")

        def warm(n):
            # Filler matmuls on scratch: keep the PE active so HAM never
            # re-throttles to 1.2GHz while waiting on DMA.
            for _ in range(n):
                nc.tensor.matmul(
                    wu[:R, :], scratch[:], scratch[:, :R], start=True,
                    stop=True, skip_group_check=True,
                )

        warm(N_WARM1)

        # Software pipeline: A/B two slices ahead, C one ahead, D current.
        # C(s+1) goes before D(s) so D(s+1) never waits on the rc copy.
        stage_A(0)
        stage_B(0)
        stage_A(1)
        stage_B(1)
        stage_C(0)
        warm(N_WARM2)
        for s in range(S):
            if s + 2 < S:
                stage_A(s + 2)
                stage_B(s + 2)
            if s + 1 < S:
                stage_C(s + 1)
            stage_D(s, fine=(s == S - 1))

    nc.compile()
    return nc


def _band_corr(taps: np.ndarray, n: int) -> np.ndarray:
    """G[i, j] = taps[j - i + PAD]: cross-correlation (matches NN conv)."""
    M = np.zeros((n, n), np.float64)
    idx = np.arange(n)
    for d in range(-PAD, PAD + 1):
        i = idx[(idx + d >= 0) & (idx + d < n)]
        M[i, i + d] = taps[d + PAD]
    return M


def _factors(taps: np.ndarray):
    """Rank-R factorization G ~= U_ @ D_ of the 1D blur matrix for `taps`."""
    G = _band_corr(taps, H)
    U, Sv, Vt = np.linalg.svd(G)
    D_ = (np.sqrt(Sv[:R])[:, None] * Vt[:R]).astype(np.float32)  # [R, 512]
    U_ = (U[:, :R] * np.sqrt(Sv[:R])).astype(np.float32)         # [512, R]
    return D_, U_


def kernel(x: np.ndarray, kernel: np.ndarray) -> np.ndarray:
    x = np.asarray(x, dtype=np.float32)
    k2d = np.asarray(kernel, dtype=np.float64)
    n, c, h, w = x.shape
    assert (h, w) == (H, W) and k2d.shape == (KS, KS)

    # Separable decomposition; the Gaussian (and all-ones) kernel is rank-1.
    U, Sv, Vt = np.linalg.svd(k2d)
    assert Sv[1] <= Sv[0] * 1e-6, "kernel not rank-1 separable"
    kx = Sv[0] * U[:, 0]  # taps along H
    ky = Vt[0]            # taps along W

    Dh, Uh = _factors(kx)
    Dw, Uw = _factors(ky)

    c128 = np.empty((PT, 2, NT, R), np.float32)
    c128[:, 0] = Dh.T.reshape(NT, PT, R).transpose(1, 0, 2)  # DhT[128*tk+p, m]
    c128[:, 1] = Dw.T.reshape(NT, PT, R).transpose(1, 0, 2)
    c64 = np.empty((R, 2, W), np.float32)
    c64[:, 0] = Uh.T  # UhT[m, h']
    c64[:, 1] = Uw.T  # UwT[n, w']
    c128 = c128.astype(BF)
    c64 = c64.astype(BF)

    if "k" not in _kernel_cache:
        _kernel_cache["k"] = _build_bass()
    nc = _kernel_cache["k"]

    # Partition-major swizzle: x_swz[s, p, tk, w] = x[s, tk*128+p, w]
    xr = x.reshape(n * c, NT, PT, W).transpose(0, 2, 1, 3).astype(BF)
    per = xr.shape[0] // N_CORES
    in_maps = [
        {
            "x": np.ascontiguousarray(xr[ci * per : (ci + 1) * per]),
            "c128": c128,
            "c64": c64,
        }
        for ci in range(N_CORES)
    ]
    res = run_bass_kernel_spmd(nc, in_maps, core_ids=list(range(N_CORES)))
    global last_results
    last_results = res
    y = np.concatenate([res.results[ci]["y"] for ci in range(N_CORES)], axis=0)
    # y[s, p, hs, w] -> [s, hs*128+p, w]
    y = y.transpose(0, 2, 1, 3).reshape(n, c, h, w).astype(np.float32)
    return y


last_results = None
